# revision 6
# baseline (speedup 1.0000x reference)
"""MoE gating kernel for Trainium2 (Bass/Tile), data-parallel over 8 NeuronCores.

Computes: logits = x @ W_g.T ; top-2 values; softmax over the 2 values.
  p1 = sigmoid(v1 - v2), p2 = sigmoid(v2 - v1)  (v1 >= v2 the top-2 logits)

Sharding: tokens split 8 ways (2048 tokens/core), W_g replicated.

v2 (bf16 pipeline): x is cast fp32->bf16 during the HBM->SBUF DMA (SWDGE),
so the whole on-chip pipeline runs 16-bit:
  - PE transposes x into xT [128 d, 16k * 256 t] in bf16 (FWL halves the
    per-transpose LDWEIGHTS cost vs fp32/f32r)
  - transposes write bf16 PSUM; 4 k-slices share one 2KB bank so each DVE
    drain moves [128, 1024] bf16 at the 2x 16-bit rate
  - 16 accumulating bf16 matmuls (N=256) -> logitsT [64 e, 256 t] fp32
  - drain + PE-transpose back to [128 t, 64 e], DVE Max8 top-2, ACT sigmoid
bf16 adds ~2e-3 max abs error on the output probabilities (gate is 2e-2).
"""

import sys

sys.path.insert(0, "/opt/trn_rl_repo")

from contextlib import ExitStack

import numpy as np

import concourse.bass as bass
import concourse.bacc as bacc
import concourse.mybir as mybir
from concourse import masks
from concourse.tile import TileContext
from concourse.bass_utils import run_bass_kernel_spmd

TOKENS = 16384
DIM = 2048
E = 64  # num experts
NCORES = 8
TPC = TOKENS // NCORES  # tokens per core
P = 128
KT = DIM // P  # 16 contraction tiles
G = 256  # token group (moving-dim of the big matmul)
NG = TPC // G  # 8 groups per core
TB = G // P  # 2 token blocks per group
KPB = 4  # k-slices packed per PSUM bank for transpose drains

F32 = mybir.dt.float32
BF16 = mybir.dt.bfloat16
N_WARM = 18  # warm-up matmuls (N=256): ~4us of PE activity flips HAM to 8/8


def _emit(tc: TileContext, ctx: ExitStack, x_ap, wg_ap, out_ap):
    nc = tc.nc

    singles = ctx.enter_context(tc.tile_pool(name="singles", bufs=1))
    xpool = ctx.enter_context(tc.tile_pool(name="xpool", bufs=1))
    xtpool = ctx.enter_context(tc.tile_pool(name="xtpool", bufs=3))
    ltpool = ctx.enter_context(tc.tile_pool(name="ltpool", bufs=2))
    spool = ctx.enter_context(tc.tile_pool(name="spool", bufs=4))
    opool = ctx.enter_context(tc.tile_pool(name="opool", bufs=4))
    psum_t = ctx.enter_context(tc.tile_pool(name="psum_t", bufs=3, space="PSUM"))
    psum_l = ctx.enter_context(tc.tile_pool(name="psum_l", bufs=2, space="PSUM"))
    psum_f = ctx.enter_context(tc.tile_pool(name="psum_f", bufs=2, space="PSUM"))
    psum_w = ctx.enter_context(tc.tile_pool(name="psum_w", bufs=1, space="PSUM"))

    # PE warm-up: dummy matmuls keep the PE busy from engine boot until the
    # first x data lands (~10us), so the HAM clock gate is at 2.4GHz (not the
    # cold 1.2GHz) when real transposes start. Memsets go on the scalar
    # engine (short preamble) so the chain starts as early as possible.
    warm = singles.tile([P, P], BF16)
    nc.gpsimd.memset(warm[:], 0.0)
    warm_rhs = singles.tile([P, 2 * P], BF16)
    nc.gpsimd.memset(warm_rhs[:], 0.0)
    for _ in range(N_WARM):
        pw = psum_w.tile([P, 2 * P], F32, tag="warm_ps")
        nc.tensor.matmul(pw[:], warm[:], warm_rhs[:])

    def ham_tickle():
        # HAM (the PE clock gate) does not count transpose-mode matmuls as
        # activity; a long pure-transpose phase re-throttles the PE to
        # 1.2GHz. A tiny regular matmul every ~1us keeps K=8/8 latched.
        pw = psum_w.tile([P, E], F32, tag="warm_ps")
        nc.tensor.matmul(pw[:], warm[:], warm_rhs[:, :E])

    # identity built before the long SWDGE descriptor-emission stream for
    # the x loads below occupies gpsimd.
    ident = singles.tile([P, P], BF16)
    masks.make_identity(nc, ident[:])

    # preload ALL of this core's x into SBUF as bf16 (cast during DMA, SWDGE);
    # HBM reads are unchanged fp32 so the DMA roofline is unaffected, but all
    # downstream engines run 16-bit. W_g is loaded after the first group's
    # tiles so it lands before build_wgT needs it (~7us) without delaying
    # the PE's first transposes.
    all_x = []
    for t in range(NG * TB):
        xt_in = xpool.tile([P, DIM], BF16, tag=f"x{t}")
        all_x.append(xt_in)
    wg_sb = singles.tile([E, DIM], BF16)
    for t in range(NG * TB):
        nc.gpsimd.dma_start(out=all_x[t][:], in_=x_ap[t * P : (t + 1) * P, :])
        if t == TB - 1:
            nc.gpsimd.dma_start(out=wg_sb[:], in_=wg_ap)

    # wgT[:, k, :] = W_g[:, k*128:(k+1)*128].T  -> [128 d, 64 e] per k-tile
    wgT = singles.tile([P, KT, E], BF16)

    def build_wgT():
        for k in range(KT):
            pt = psum_f.tile([P, E], BF16, tag="fin_ps")
            nc.tensor.matmul(
                pt[:],
                wg_sb[:, k * P : (k + 1) * P],
                ident[:E, :E],
                is_transpose=True,
            )
            nc.vector.tensor_copy(wgT[:, k, :], pt[:])

    def epilogue(g, lp):
        # back to token-major + top-2 + softmax (runs one group late)
        lt = ltpool.tile([E, G], F32)
        for tb in range(TB):
            nc.vector.tensor_copy(
                lt[:, tb * P : (tb + 1) * P], lp[:, tb * P : (tb + 1) * P]
            )
            fp = psum_f.tile([P, E], F32, tag="fin_ps")
            nc.tensor.matmul(
                fp[:],
                lt[:, tb * P : (tb + 1) * P],
                ident_f[:E, :E],
                is_transpose=True,
            )
            max8 = spool.tile([P, 8], F32)
            nc.vector.max(out=max8[:], in_=fp[:])
            dd = spool.tile([P, 2], F32)
            nc.vector.tensor_sub(dd[:, 0:1], max8[:, 0:1], max8[:, 1:2])  # v1-v2
            nc.vector.tensor_sub(dd[:, 1:2], max8[:, 1:2], max8[:, 0:1])  # v2-v1
            ot = opool.tile([P, 2], F32)
            nc.scalar.activation(ot[:], dd[:], mybir.ActivationFunctionType.Sigmoid)
            r0 = g * G + tb * P
            nc.sync.dma_start(out=out_ap[r0 : r0 + P, :], in_=ot[:])

    # fp32 identity for the (fp32) epilogue transpose-back
    ident_f = singles.tile([P, P], F32)
    masks.make_identity(nc, ident_f[:])

    pending = None  # (g, lp) awaiting epilogue
    for g in range(NG):
        xtiles = all_x[g * TB : (g + 1) * TB]

        # transpose into xT [128 d, k * G t] (bf16). KPB k-tiles share one
        # 2KB PSUM bank so each DVE drain covers [128, KPB*G] at the 16-bit
        # 2x rate.
        xt = xtpool.tile([P, KT * G], BF16)
        for q in range(KT // KPB):
            pt = psum_t.tile([P, KPB * G], BF16)
            for dk in range(KPB):
                k = KPB * q + dk
                for tb in range(TB):
                    nc.tensor.matmul(
                        pt[:, dk * G + tb * P : dk * G + (tb + 1) * P],
                        xtiles[tb][:, k * P : (k + 1) * P],
                        ident[:],
                        is_transpose=True,
                    )
            nc.vector.tensor_copy(xt[:, q * KPB * G : (q + 1) * KPB * G], pt[:])
            ham_tickle()

        if g == 0:
            build_wgT()

        # logitsT [64 e, 256 t] = sum_k wgT_k.T @ xT_k  (bf16 -> fp32 PSUM)
        lp = psum_l.tile([E, G], F32)
        for k in range(KT):
            nc.tensor.matmul(
                lp[:],
                wgT[:, k, :],
                xt[:, k * G : (k + 1) * G],
                start=(k == 0),
                stop=(k == KT - 1),
            )

        if pending is not None:
            epilogue(*pending)
        pending = (g, lp)
    epilogue(*pending)


_NC_CACHE = {}


def _build():
    key = "nc"
    if key in _NC_CACHE:
        return _NC_CACHE[key]
    nc = bacc.Bacc(trn_type="TRN2")
    x = nc.dram_tensor("x", [TPC, DIM], F32, kind="ExternalInput")
    wg = nc.dram_tensor("w_g", [E, DIM], F32, kind="ExternalInput")
    out = nc.dram_tensor("out", [TPC, 2], F32, kind="ExternalOutput")
    with TileContext(nc) as tc, ExitStack() as ctx:
        _emit(tc, ctx, x.ap(), wg.ap(), out.ap())
    if not nc.is_finalized():
        nc.finalize()
    _NC_CACHE[key] = nc
    return nc


def _run(x, W_g, trace=False):
    nc = _build()
    x = np.ascontiguousarray(np.asarray(x, dtype=np.float32))
    W_g = np.ascontiguousarray(np.asarray(W_g, dtype=np.float32))
    in_maps = [
        {"x": np.ascontiguousarray(x[c * TPC : (c + 1) * TPC]), "w_g": W_g}
        for c in range(NCORES)
    ]
    res = run_bass_kernel_spmd(nc, in_maps, core_ids=list(range(NCORES)), trace=trace)
    out = np.concatenate([r["out"] for r in res.results], axis=0)
    return out, res


def kernel(x, W_g):
    out, _ = _run(x, W_g, trace=False)
    return out


def kernel_profiled(x, W_g, **_kw):
    out, res = _run(x, W_g, trace=True)
    return out, res


# revision 7
# speedup vs baseline: 1.0844x; 1.0844x over previous
"""MoE gating kernel for Trainium2 (Bass/Tile), data-parallel over 8 NeuronCores.

Computes: logits = x @ W_g.T ; top-2 values; softmax over the 2 values.
  p1 = sigmoid(v1 - v2), p2 = sigmoid(v2 - v1)  (v1 >= v2 the top-2 logits)

Sharding: tokens split 8 ways (2048 tokens/core), W_g replicated.

v4 design notes:
  - x is cast fp32->bf16 during the HBM->SBUF DMA (SWDGE). HBM reads are
    unchanged, so the ~45us DMA stream (the roofline) is unaffected, but the
    on-chip pipeline runs 16-bit: FWL halves the per-transpose LDWEIGHTS
    cost and bf16 matmuls feed 2KB PSUM banks efficiently.
  - transposes are REGULAR matmuls against an identity moving operand
    (out = x_block.T @ I), NOT transpose-mode: the HAM clock gate does not
    count transpose-mode passes as PE activity, and an uncounted-activity
    phase longer than one HAM window re-throttles the PE to 1.2GHz (this
    cost v2/v3 ~30-40us of half-clock time). Regular matmuls keep K=8/8
    latched. Their PSUM output is fp32 (TRN2 rule), so drains run at the
    DVE 1x rate -> they are split between DVE and the otherwise-idle ACT.
  - sigmoids are batched into one ACT call at the end so ACT's activation
    table never switches between Copy and Sigmoid mid-kernel; the output
    is written by one strided DMA.
bf16 adds ~4e-3 relative error on the output probabilities (gate is 2e-2).
"""

import sys

sys.path.insert(0, "/opt/trn_rl_repo")

from contextlib import ExitStack

import numpy as np

import concourse.bass as bass
import concourse.bacc as bacc
import concourse.mybir as mybir
from concourse import masks
from concourse.tile import TileContext
from concourse.bass_utils import run_bass_kernel_spmd

TOKENS = 16384
DIM = 2048
E = 64  # num experts
NCORES = 8
TPC = TOKENS // NCORES  # tokens per core
P = 128
KT = DIM // P  # 16 contraction tiles
G = 256  # token group (moving-dim of the big matmul)
NG = TPC // G  # 8 groups per core
TB = G // P  # 2 token blocks per group
NB = NG * TB  # 16 token blocks per core

F32 = mybir.dt.float32
BF16 = mybir.dt.bfloat16
N_WARM = 18  # warm-up matmuls (N=256): ~4us of PE activity flips HAM to 8/8


def _emit(tc: TileContext, ctx: ExitStack, x_ap, wg_ap, out_ap):
    nc = tc.nc

    singles = ctx.enter_context(tc.tile_pool(name="singles", bufs=1))
    xpool = ctx.enter_context(tc.tile_pool(name="xpool", bufs=1))
    xtpool = ctx.enter_context(tc.tile_pool(name="xtpool", bufs=3))
    ltpool = ctx.enter_context(tc.tile_pool(name="ltpool", bufs=2))
    spool = ctx.enter_context(tc.tile_pool(name="spool", bufs=4))
    psum_t = ctx.enter_context(tc.tile_pool(name="psum_t", bufs=4, space="PSUM"))
    psum_l = ctx.enter_context(tc.tile_pool(name="psum_l", bufs=2, space="PSUM"))
    psum_f = ctx.enter_context(tc.tile_pool(name="psum_f", bufs=2, space="PSUM"))

    # PE warm-up: dummy matmuls keep the PE busy from engine boot until the
    # first x data lands (~10us), so the HAM clock gate is at 2.4GHz (not
    # the cold 1.2GHz) when real work starts.
    warm = singles.tile([P, P], BF16)
    nc.gpsimd.memset(warm[:], 0.0)
    warm_rhs = singles.tile([P, 2 * P], BF16)
    nc.gpsimd.memset(warm_rhs[:], 0.0)
    for _ in range(N_WARM):
        pw = psum_f.tile([P, 2 * P], F32, tag="fin_ps")
        nc.tensor.matmul(pw[:], warm[:], warm_rhs[:])

    # identity built before the long SWDGE descriptor-emission stream for
    # the x loads below occupies gpsimd.
    ident = singles.tile([P, P], BF16)
    masks.make_identity(nc, ident[:])
    ident_f = singles.tile([P, P], F32)
    masks.make_identity(nc, ident_f[:])

    # preload ALL of this core's x into SBUF as bf16 (cast during DMA,
    # SWDGE). W_g is loaded after the first group's tiles so it lands
    # before build_wgT needs it without delaying the first transposes.
    all_x = []
    for t in range(NB):
        xt_in = xpool.tile([P, DIM], BF16, tag=f"x{t}")
        all_x.append(xt_in)
    wg_sb = singles.tile([E, DIM], BF16)
    for t in range(NB):
        nc.gpsimd.dma_start(out=all_x[t][:], in_=x_ap[t * P : (t + 1) * P, :])
        if t == TB - 1:
            nc.gpsimd.dma_start(out=wg_sb[:], in_=wg_ap)

    # wgT[:, k, :] = W_g[:, k*128:(k+1)*128].T  -> [128 d, 64 e] per k-tile
    wgT = singles.tile([P, KT, E], BF16)

    def build_wgT():
        for k in range(KT):
            pt = psum_f.tile([P, E], F32, tag="fin_ps")
            # regular matmul against identity = transpose (HAM-countable)
            nc.tensor.matmul(pt[:], wg_sb[:, k * P : (k + 1) * P], ident[:E, :E])
            nc.vector.tensor_copy(wgT[:, k, :], pt[:])

    # per-token-block v1-v2 / v2-v1 accumulate here; one sigmoid + one
    # strided out-DMA at the end (keeps ACT's table in Copy mode all run).
    dd_all = singles.tile([P, NB, 2], F32)

    def epilogue(g, lp):
        # back to token-major + top-2 (runs one group late)
        lt = ltpool.tile([E, G], F32)
        for tb in range(TB):
            nc.vector.tensor_copy(
                lt[:, tb * P : (tb + 1) * P], lp[:, tb * P : (tb + 1) * P]
            )
            fp = psum_f.tile([P, E], F32, tag="fin_ps")
            nc.tensor.matmul(
                fp[:],
                lt[:, tb * P : (tb + 1) * P],
                ident_f[:E, :E],
                is_transpose=True,
            )
            max8 = spool.tile([P, 8], F32)
            nc.vector.max(out=max8[:], in_=fp[:])
            b = g * TB + tb
            nc.vector.tensor_sub(dd_all[:, b, 0:1], max8[:, 0:1], max8[:, 1:2])
            nc.vector.tensor_sub(dd_all[:, b, 1:2], max8[:, 1:2], max8[:, 0:1])

    pending = None  # (g, lp) awaiting epilogue
    for g in range(NG):
        xtiles = all_x[g * TB : (g + 1) * TB]

        # transpose into xT [128 d, k * G t] (bf16 in SBUF, fp32 in PSUM).
        # Each 2KB PSUM bank holds 2 k-slices x 2 token blocks; drains
        # alternate DVE / ACT so neither engine becomes critical.
        xt = xtpool.tile([P, KT * G], BF16)
        for q in range(KT // 2):
            pt = psum_t.tile([P, 2 * G], F32)
            for dk in range(2):
                k = 2 * q + dk
                for tb in range(TB):
                    nc.tensor.matmul(
                        pt[:, dk * G + tb * P : dk * G + (tb + 1) * P],
                        xtiles[tb][:, k * P : (k + 1) * P],
                        ident[:],
                    )
            dst = xt[:, 2 * q * G : (2 * q + 2) * G]
            if q % 2 == 0:
                nc.vector.tensor_copy(dst, pt[:])
            else:
                nc.scalar.copy(dst, pt[:])

        if g == 0:
            build_wgT()

        # logitsT [64 e, 256 t] = sum_k wgT_k.T @ xT_k  (bf16 -> fp32 PSUM)
        lp = psum_l.tile([E, G], F32)
        for k in range(KT):
            nc.tensor.matmul(
                lp[:],
                wgT[:, k, :],
                xt[:, k * G : (k + 1) * G],
                start=(k == 0),
                stop=(k == KT - 1),
            )

        if pending is not None:
            epilogue(*pending)
        pending = (g, lp)
    epilogue(*pending)

    # single sigmoid + single strided store: out[b*128+p, c] = ot[p, b, c]
    ot = singles.tile([P, NB, 2], F32)
    nc.scalar.activation(ot[:], dd_all[:], mybir.ActivationFunctionType.Sigmoid)
    nc.sync.dma_start(
        out=out_ap.rearrange("(b p) c -> p b c", p=P),
        in_=ot[:],
    )


_NC_CACHE = {}


def _build():
    key = "nc"
    if key in _NC_CACHE:
        return _NC_CACHE[key]
    nc = bacc.Bacc(trn_type="TRN2")
    x = nc.dram_tensor("x", [TPC, DIM], F32, kind="ExternalInput")
    wg = nc.dram_tensor("w_g", [E, DIM], F32, kind="ExternalInput")
    out = nc.dram_tensor("out", [TPC, 2], F32, kind="ExternalOutput")
    with TileContext(nc) as tc, ExitStack() as ctx:
        _emit(tc, ctx, x.ap(), wg.ap(), out.ap())
    if not nc.is_finalized():
        nc.finalize()
    _NC_CACHE[key] = nc
    return nc


def _run(x, W_g, trace=False):
    nc = _build()
    x = np.ascontiguousarray(np.asarray(x, dtype=np.float32))
    W_g = np.ascontiguousarray(np.asarray(W_g, dtype=np.float32))
    in_maps = [
        {"x": np.ascontiguousarray(x[c * TPC : (c + 1) * TPC]), "w_g": W_g}
        for c in range(NCORES)
    ]
    res = run_bass_kernel_spmd(nc, in_maps, core_ids=list(range(NCORES)), trace=trace)
    out = np.concatenate([r["out"] for r in res.results], axis=0)
    return out, res


def kernel(x, W_g):
    out, _ = _run(x, W_g, trace=False)
    return out


def kernel_profiled(x, W_g, **_kw):
    out, res = _run(x, W_g, trace=True)
    return out, res
